# revision 1
# baseline (speedup 1.0000x reference)
"""Trainium2 Bass kernel for nn_Conv2d_72052371357971.

Text-CNN style conv stack: three conv groups (k=1,2,3) over [N,3,256]
windows + per-group max-pool, concatenated to [N,256].

Strategy (pure data parallel across 8 NeuronCores):
  * All three conv groups fold into ONE [768, 406] weight matrix over the
    flattened window (3*256 channels). Group outputs needing max-pooling
    occupy disjoint column ranges; pooling is an elementwise max of column
    slices afterwards.
  * Host repacks x into channel-major [128, batch] tiles (free: only device
    time counts) so the contraction dim sits on SBUF partitions.
  * Device, per 128-row batch tile: 7 accumulating matmuls into one PSUM
    bank (6 K-subtiles of 128 + a K=1 ones-row matmul that adds the bias),
    ScalarE copies PSUM->SBUF, VectorE does the pools, DMA streams out
    [batch, 256] rows.
  * DMA is batched into 1024-row super-tiles (1.5 MB loads / 1 MB stores).
"""

import numpy as np

import concourse.bacc as bacc
import concourse.mybir as mybir
import concourse.tile as tile
from concourse.bass import ds
from concourse.bass_utils import run_bass_kernel_spmd

# Problem shapes (hardcoded per contract)
N = 65536
NCORES = 8
B = N // NCORES           # 8192 batch rows per core
TB = 128                  # batch tile (PSUM partition dim)
TPS = 8                   # batch tiles per super-tile
SUP = B // (TPS * TB)     # 8 super-tiles per core
K = 768                   # contraction: 3 positions x 256 channels
KS = K // 128             # 6 K-subtiles
F = 406                   # pre-pool filters: 3*50 + 2*50 + 156
FO = 256                  # output filters after pooling

_F32 = mybir.dt.float32
# matmul operand dtype: float32r streams at 1 col/cycle (vs 4 for float32)
# on the trn2 PE when the moving free dim is >=256; same 4-byte fp32 bits.
_F32R = mybir.dt.float32r
_cache = {}


def _build_nc(
    reps=1,
    has_bias=True,
    xbufs=2,
    obufs=2,
    ybufs=8,
    pbufs=8,
    pad512=False,  # timing diagnostic: stream 512 weight cols per matmul
    dupx=False,  # timing diagnostic: load x twice per super
    dvepsum=False,  # DVE copies the o3 slice straight from PSUM; ACT copies only 250 cols
    trim=False,  # block-sparse column order [A D F E C B]: stream only nonzero spans
):
    FF = 512 if pad512 else F
    nc = bacc.Bacc("TRN2", target_bir_lowering=False, debug=False)

    x_d = nc.dram_tensor("x", [SUP, 128, TPS * KS * TB], _F32R, kind="ExternalInput")
    w_d = nc.dram_tensor("w", [128, KS * FF], _F32R, kind="ExternalInput")
    # bias row and a ones row (walrus rejects memset on float32r tiles, so
    # the ones come from DRAM; same 4-byte payload as float32)
    b_d = nc.dram_tensor("b", [1, F + TB], _F32R, kind="ExternalInput")
    o_d = nc.dram_tensor("o", [SUP, TPS, TB, FO], _F32, kind="ExternalOutput")

    with tile.TileContext(nc) as tc:
        with (
            tc.tile_pool(name="const", bufs=1) as constp,
            tc.tile_pool(name="xp", bufs=xbufs) as xp,
            tc.tile_pool(name="yp", bufs=ybufs) as yp,
            tc.tile_pool(name="op", bufs=obufs) as op,
            tc.tile_pool(name="ps", bufs=pbufs, space="PSUM") as psp,
        ):
            wt = constp.tile([128, KS * FF], _F32R)
            nc.sync.dma_start(wt[:], w_d[:])
            if has_bias:
                bt = constp.tile([1, F + TB], _F32R)
                nc.sync.dma_start(bt[:], b_d[:])
                brow = bt[:, ds(0, F)]
                ones = bt[:, ds(F, TB)]

            for s in [si for _ in range(reps) for si in range(SUP)]:
                xt = xp.tile([128, TPS * KS * TB], _F32R)
                # one whole-super load measured faster than split halves on HW
                nc.sync.dma_start(xt[:], x_d[s])
                if dupx:
                    xt2 = xp.tile([128, TPS * KS * TB], _F32R, tag="xdup")
                    nc.sync.dma_start(xt2[:], x_d[s])
                    nc.vector.tensor_copy(xt[:, ds(0, 4)], xt2[:, ds(0, 4)])
                ot = op.tile([128, TPS * FO], _F32)
                # (j, col0, ncols, start): trim streams only each token's
                # nonzero span; the full-width j=2 goes first with start=True
                # so it zero-fills the columns later matmuls never touch.
                if trim:
                    spans = [
                        (2, 0, 406, True),
                        (3, 50, 356, False),
                        (0, 0, 256, False),
                        (1, 0, 256, False),
                        (4, 100, 256, False),
                        (5, 100, 256, False),
                    ]
                else:
                    spans = [(j, 0, FF, j == 0) for j in range(KS)]
                for t in range(TPS):
                    acc = psp.tile([128, FF], _F32)
                    for idx, (j, c0, w, st) in enumerate(spans):
                        nc.tensor.matmul(
                            acc[:, ds(c0, w)],
                            lhsT=xt[:, ds(t * KS * TB + j * TB, TB)],
                            rhs=wt[:, ds(j * FF + c0, w)],
                            start=st,
                            stop=(idx == KS - 1) and not has_bias,
                        )
                    if has_bias:
                        nc.tensor.matmul(
                            acc[:], lhsT=ones, rhs=brow, start=False, stop=True
                        )
                    ycols = 250 if (dvepsum and not trim) else F
                    y = yp.tile([128, ycols], _F32)
                    nc.scalar.activation(
                        y[:], acc[:, ds(0, ycols)], mybir.ActivationFunctionType.Copy
                    )
                    o0 = t * FO
                    # column positions of groups A,B,C (o1) / D,E (o2) / F (o3)
                    (ca, cb, cc, cd, ce, cf) = (
                        (0, 356, 306, 50, 256, 100)
                        if trim
                        else (0, 50, 100, 150, 200, 250)
                    )
                    nc.vector.tensor_max(
                        ot[:, ds(o0, 50)], y[:, ds(ca, 50)], y[:, ds(cb, 50)]
                    )
                    nc.vector.tensor_max(
                        ot[:, ds(o0, 50)], ot[:, ds(o0, 50)], y[:, ds(cc, 50)]
                    )
                    nc.vector.tensor_max(
                        ot[:, ds(o0 + 50, 50)], y[:, ds(cd, 50)], y[:, ds(ce, 50)]
                    )
                    nc.vector.tensor_copy(
                        ot[:, ds(o0 + 100, 156)],
                        (acc if dvepsum else y)[:, ds(cf, 156)],
                    )
                # SBUF [p, (t f)] -> DRAM [t, p, f]
                nc.sync.dma_start(
                    o_d[s].rearrange("t p f -> p t f"),
                    ot[:].rearrange("p (t f) -> p t f", t=TPS),
                )
    nc.compile()
    return nc


def _pack_weights(W1, b1, W2, b2, W3, b3, trim=False):
    Wc = np.zeros((K, F), np.float32)
    if trim:
        # column order [A D F E C B] so each token's nonzero cols form one span
        Wc[0:256, 0:50] = W1.T  # A = y1h0
        Wc[0:256, 50:100] = W2[:, 0, :].T  # D = y2h0
        Wc[256:512, 50:100] = W2[:, 1, :].T
        Wc[:, 100:256] = W3.reshape(156, K).T  # F = o3
        Wc[256:512, 256:306] = W2[:, 0, :].T  # E = y2h1
        Wc[512:768, 256:306] = W2[:, 1, :].T
        Wc[512:768, 306:356] = W1.T  # C = y1h2
        Wc[256:512, 356:406] = W1.T  # B = y1h1
        bparts = [b1[:, 0], b2[:, 0], b3, b2[:, 1], b1[:, 2], b1[:, 1]]
    else:
        Wc[0:256, 0:50] = W1.T
        Wc[256:512, 50:100] = W1.T
        Wc[512:768, 100:150] = W1.T
        Wc[0:256, 150:200] = W2[:, 0, :].T
        Wc[256:512, 150:200] = W2[:, 1, :].T
        Wc[256:512, 200:250] = W2[:, 0, :].T
        Wc[512:768, 200:250] = W2[:, 1, :].T
        Wc[:, 250:406] = W3.reshape(156, K).T
        bparts = [b1[:, 0], b1[:, 1], b1[:, 2], b2[:, 0], b2[:, 1], b3]
    wt = np.ascontiguousarray(
        Wc.reshape(KS, 128, F).transpose(1, 0, 2).reshape(128, KS * F)
    )
    brow = np.concatenate(bparts + [np.ones(TB)]).astype(np.float32)[None, :]
    return wt, brow


def kernel(x, W1, b1, W2, b2, W3, b3):
    x = np.ascontiguousarray(x, np.float32)
    wt, brow = _pack_weights(
        np.asarray(W1, np.float32),
        np.asarray(b1, np.float32),
        np.asarray(W2, np.float32),
        np.asarray(b2, np.float32),
        np.asarray(W3, np.float32),
        np.asarray(b3, np.float32),
    )

    has_bias = bool(np.any(brow[:, :F] != 0.0))
    key = ("nc", has_bias)
    if key not in _cache:
        _cache[key] = _build_nc(has_bias=has_bias)
    nc = _cache[key]

    xs = x.reshape(N, K)
    in_maps = []
    for c in range(NCORES):
        xc = xs[c * B : (c + 1) * B]
        # [s, t, f, j, p] -> [s, p, t, j, f] so each super-tile is one
        # contiguous [128, TPS*KS*TB] channel-major block
        arr = np.ascontiguousarray(
            xc.reshape(SUP, TPS, TB, KS, 128).transpose(0, 4, 1, 3, 2)
        ).reshape(SUP, 128, TPS * KS * TB)
        in_maps.append({"x": arr, "w": wt, "b": brow})

    res = run_bass_kernel_spmd(nc, in_maps, list(range(NCORES)))

    outs = []
    for c in range(NCORES):
        o = res.results[c]["o"]  # [SUP, TPS, TB, FO]; (s,t,p) == batch order
        outs.append(np.asarray(o).reshape(B, FO))
    out = np.concatenate(outs, axis=0)
    return out[:, :, None, None]



# revision 13
# speedup vs baseline: 19.4886x; 19.4886x over previous
"""Trainium2 Bass kernel for nn_Conv2d_72052371357971.

Text-CNN style conv stack: three conv groups (k=1,2,3) over [N,3,256]
windows + per-group max-pool, concatenated to [N,256].

Strategy (pure data parallel across 8 NeuronCores):
  * All three conv groups fold into ONE [768, 406] weight matrix over the
    flattened window (3*256 channels). Group outputs needing max-pooling
    occupy disjoint column ranges; pooling is an elementwise max of column
    slices afterwards.
  * Host repacks x into channel-major [128, batch] tiles and converts to
    bf16 (host work is free: only device time counts) so the contraction
    dim sits on SBUF partitions and DMA traffic is halved.
  * Weight columns are laid out block-sparse ([A D F E C B]) so each
    token's nonzero filter columns form one contiguous span; matmuls
    stream only those spans (1786 cols/tile vs 2436 dense), cutting PE
    streaming time 27%.
  * Device, per 128-row batch tile: 6 accumulating bf16 matmuls into one
    PSUM bank (f32 accumulate), ScalarE copies PSUM->SBUF as bf16,
    VectorE does the pools, DMA streams out contiguous [128, TPS*256]
    bf16 blocks (4 KB per partition line); host converts back to f32.
  * DMA is batched into 1024-row super-tiles (0.75 MB loads / 0.5 MB
    stores in bf16).
"""

import numpy as np
import ml_dtypes

import concourse.bacc as bacc
import concourse.mybir as mybir
import concourse.tile as tile
from concourse.bass import ds
from concourse.bass_utils import run_bass_kernel_spmd

# Problem shapes (hardcoded per contract)
N = 65536
NCORES = 8
B = N // NCORES           # 8192 batch rows per core
TB = 128                  # batch tile (PSUM partition dim)
TPS = 8                   # batch tiles per super-tile
SUP = B // (TPS * TB)     # 8 super-tiles per core
K = 768                   # contraction: 3 positions x 256 channels
KS = K // 128             # 6 K-subtiles
F = 406                   # pre-pool filters: 3*50 + 2*50 + 156
FO = 256                  # output filters after pooling

_F32 = mybir.dt.float32
# f32r streams 1 col/cycle on the trn2 PE only when the moving free dim is
# >=256; bf16 streams 1 col/cycle unconditionally and halves DMA traffic.
_F32R = mybir.dt.float32r
_BF16 = mybir.dt.bfloat16
BF16NP = np.dtype(ml_dtypes.bfloat16)
_cache = {}

# Block-sparse weight-column spans per K-subtile j (trim layout
# [A(0:50) D(50:100) F(100:256) E(256:306) C(306:356) B(356:406)]):
# (j, col0, ncols, start).  The full-width j=2 span goes first with
# start=True so it zero-fills the columns later matmuls never touch.
TRIM_SPANS = [
    (2, 0, 406, True),
    (3, 50, 356, False),
    (0, 0, 256, False),
    (1, 0, 256, False),
    (4, 100, 256, False),
    (5, 100, 256, False),
]
# Minimal-stream schedule: 1636 cols/tile, the exact nonzero structure.
# PSUM start_tensor_calc marks the whole 2KB bank pending-zero (interp
# ZERO_REGION_SIZE), so exactly ONE start per accumulation group; later
# matmuls auto-zero on their first touch of pending bytes, but each matmul
# must touch a uniformly pending or uniformly written region -- hence the
# splits at the touched/untouched boundaries.  More matmuls -> more
# stationary weight loads; wins only if LD_WEIGHTS overlaps streaming.
SPANS_MIN = [
    (0, 0, 256, True),    # A,D,F := j0; bank pending elsewhere
    (2, 256, 50, False),  # E := j2 (first touch)
    (1, 0, 256, False),   # A,D,F += j1
    (2, 50, 206, False),  # D,F += j2
    (3, 50, 256, False),  # D,F,E += j3
    (4, 306, 50, False),  # C := j4 (first touch)
    (5, 100, 256, False),  # F,E,C += j5
    (4, 100, 206, False),  # F,E += j4
    (2, 356, 50, False),  # B := j2 (first touch)
    (3, 356, 50, False),  # B += j3
]
# column positions of groups A,B,C (o1) / D,E (o2) / F (o3) per layout
TRIM_POOLCOLS = (0, 356, 306, 50, 256, 100)
DENSE_POOLCOLS = (0, 50, 100, 150, 200, 250)


def _build_nc(
    reps=1,
    has_bias=True,
    dt="bf16",  # matmul/IO operand dtype: "bf16" or "f32r"
    trim=True,  # stream only each token's nonzero column span
    spans9=True,  # minimal 1636-col 10-matmul schedule (implies trim layout)
    prefetch=True,  # issue load of super s+1 before store of super s
    store_q="act",  # engine whose DGE queue issues output stores
    xbufs=3,
    obufs=2,
    ybufs=8,
    pbufs=8,
):
    mdt = _BF16 if dt == "bf16" else _F32R
    odt = _BF16 if dt == "bf16" else _F32
    nc = bacc.Bacc("TRN2", target_bir_lowering=False, debug=False)

    x_d = nc.dram_tensor("x", [SUP, 128, TPS * KS * TB], mdt, kind="ExternalInput")
    w_d = nc.dram_tensor("w", [128, KS * F], mdt, kind="ExternalInput")
    # bias row and a ones row (walrus rejects memset on f32r/bf16 tiles, so
    # the ones come from DRAM)
    b_d = nc.dram_tensor("b", [1, F + TB], mdt, kind="ExternalInput")
    o_d = nc.dram_tensor("o", [SUP, 128, TPS * FO], odt, kind="ExternalOutput")

    with tile.TileContext(nc) as tc:
        with (
            tc.tile_pool(name="const", bufs=1) as constp,
            tc.tile_pool(name="xp", bufs=xbufs) as xp,
            tc.tile_pool(name="yp", bufs=ybufs) as yp,
            tc.tile_pool(name="op", bufs=obufs) as op,
            tc.tile_pool(name="ps", bufs=pbufs, space="PSUM") as psp,
        ):
            wt = constp.tile([128, KS * F], mdt)
            nc.sync.dma_start(wt[:], w_d[:])
            if has_bias:
                bt = constp.tile([1, F + TB], mdt)
                nc.sync.dma_start(bt[:], b_d[:])
                brow = bt[:, ds(0, F)]
                ones = bt[:, ds(F, TB)]

            if spans9:
                trim = True
                spans = SPANS_MIN
            else:
                spans = TRIM_SPANS if trim else [(j, 0, F, j == 0) for j in range(KS)]
            seq = [si for _ in range(reps) for si in range(SUP)]
            # Both input loads and output stores trigger from an engine's DGE
            # queue in that engine's program order.  On one queue the load of
            # super s+1 sits behind the store of super s, whose semaphore wait
            # clears only when compute s finishes -- fully serializing DMA with
            # compute (measured: exec == PE time + DMA time).  Fix: prefetch
            # the next super's load before this super's store is issued, and
            # put stores on the Activation engine's DGE queue.
            store_eng = {"act": nc.scalar, "sp": nc.sync, "gpsimd": nc.gpsimd}[store_q]
            xts = {}

            def load(i):
                xt = xp.tile([128, TPS * KS * TB], mdt)
                # one whole-super load measured faster than split halves on HW
                nc.sync.dma_start(xt[:], x_d[seq[i]])
                xts[i] = xt

            if prefetch:
                load(0)
            for i, s in enumerate(seq):
                if prefetch:
                    xt = xts.pop(i)
                    if i + 1 < len(seq):
                        load(i + 1)
                else:
                    load(i)
                    xt = xts.pop(i)
                ot = op.tile([128, TPS * FO], odt)
                for t in range(TPS):
                    acc = psp.tile([128, F], _F32)
                    for idx, (j, c0, w, st) in enumerate(spans):
                        nc.tensor.matmul(
                            acc[:, ds(c0, w)],
                            lhsT=xt[:, ds(t * KS * TB + j * TB, TB)],
                            rhs=wt[:, ds(j * F + c0, w)],
                            start=st,
                            stop=(idx == len(spans) - 1) and not has_bias,
                        )
                    if has_bias:
                        nc.tensor.matmul(
                            acc[:], lhsT=ones, rhs=brow, start=False, stop=True
                        )
                    y = yp.tile([128, F], odt)
                    nc.scalar.activation(
                        y[:], acc[:], mybir.ActivationFunctionType.Copy
                    )
                    o0 = t * FO
                    (ca, cb, cc, cd, ce, cf) = (
                        TRIM_POOLCOLS if trim else DENSE_POOLCOLS
                    )
                    nc.vector.tensor_max(
                        ot[:, ds(o0, 50)], y[:, ds(ca, 50)], y[:, ds(cb, 50)]
                    )
                    nc.vector.tensor_max(
                        ot[:, ds(o0, 50)], ot[:, ds(o0, 50)], y[:, ds(cc, 50)]
                    )
                    nc.vector.tensor_max(
                        ot[:, ds(o0 + 50, 50)], y[:, ds(cd, 50)], y[:, ds(ce, 50)]
                    )
                    nc.vector.tensor_copy(ot[:, ds(o0 + 100, 156)], y[:, ds(cf, 156)])
                # SBUF [p, (t f)] stored contiguously; host untangles (t, p)
                store_eng.dma_start(o_d[s], ot[:])
    nc.compile()
    return nc


def _pack_weights(W1, b1, W2, b2, W3, b3, trim=True):
    Wc = np.zeros((K, F), np.float32)
    if trim:
        # column order [A D F E C B] so each token's nonzero cols form one span
        Wc[0:256, 0:50] = W1.T  # A = y1h0
        Wc[0:256, 50:100] = W2[:, 0, :].T  # D = y2h0
        Wc[256:512, 50:100] = W2[:, 1, :].T
        Wc[:, 100:256] = W3.reshape(156, K).T  # F = o3
        Wc[256:512, 256:306] = W2[:, 0, :].T  # E = y2h1
        Wc[512:768, 256:306] = W2[:, 1, :].T
        Wc[512:768, 306:356] = W1.T  # C = y1h2
        Wc[256:512, 356:406] = W1.T  # B = y1h1
        bparts = [b1[:, 0], b2[:, 0], b3, b2[:, 1], b1[:, 2], b1[:, 1]]
    else:
        Wc[0:256, 0:50] = W1.T
        Wc[256:512, 50:100] = W1.T
        Wc[512:768, 100:150] = W1.T
        Wc[0:256, 150:200] = W2[:, 0, :].T
        Wc[256:512, 150:200] = W2[:, 1, :].T
        Wc[256:512, 200:250] = W2[:, 0, :].T
        Wc[512:768, 200:250] = W2[:, 1, :].T
        Wc[:, 250:406] = W3.reshape(156, K).T
        bparts = [b1[:, 0], b1[:, 1], b1[:, 2], b2[:, 0], b2[:, 1], b3]
    wt = np.ascontiguousarray(
        Wc.reshape(KS, 128, F).transpose(1, 0, 2).reshape(128, KS * F)
    )
    brow = np.concatenate(bparts + [np.ones(TB)]).astype(np.float32)[None, :]
    return wt, brow


def _make_in_maps(inputs, dt="bf16", trim=True):
    npdt = BF16NP if dt == "bf16" else np.float32
    x = np.asarray(inputs["x"], np.float32)
    wt, brow = _pack_weights(
        np.asarray(inputs["W1"], np.float32),
        np.asarray(inputs["b1"], np.float32),
        np.asarray(inputs["W2"], np.float32),
        np.asarray(inputs["b2"], np.float32),
        np.asarray(inputs["W3"], np.float32),
        np.asarray(inputs["b3"], np.float32),
        trim=trim,
    )
    wt = wt.astype(npdt)
    brow_c = brow.astype(npdt)
    xs = x.reshape(N, K).astype(npdt)
    in_maps = []
    for c in range(NCORES):
        xc = xs[c * B : (c + 1) * B]
        # [s, t, f, j, p] -> [s, p, t, j, f] so each super-tile is one
        # contiguous [128, TPS*KS*TB] channel-major block
        arr = np.ascontiguousarray(
            xc.reshape(SUP, TPS, TB, KS, 128).transpose(0, 4, 1, 3, 2)
        ).reshape(SUP, 128, TPS * KS * TB)
        in_maps.append({"x": arr, "w": wt, "b": brow_c})
    return in_maps, bool(np.any(brow[:, :F] != 0.0))


def _unshard(per_core_results):
    outs = []
    for c in range(NCORES):
        o = np.asarray(per_core_results[c]["o"]).astype(np.float32)
        # stored [s, p, (t f)]; batch index is s*TPS*TB + t*TB + p
        o = o.reshape(SUP, 128, TPS, FO).transpose(0, 2, 1, 3).reshape(B, FO)
        outs.append(o)
    return np.concatenate(outs, axis=0)[:, :, None, None]


def kernel(x, W1, b1, W2, b2, W3, b3):
    in_maps, has_bias = _make_in_maps(
        {"x": x, "W1": W1, "b1": b1, "W2": W2, "b2": b2, "W3": W3, "b3": b3}
    )
    key = ("nc", has_bias)
    if key not in _cache:
        _cache[key] = _build_nc(has_bias=has_bias)
    nc = _cache[key]
    res = run_bass_kernel_spmd(nc, in_maps, list(range(NCORES)))
    return _unshard(res.results)
